# revision 1
# baseline (speedup 1.0000x reference)
"""Trainium2 Bass kernel for the BDH dense-transformer problem.

Shapes (hardcoded): B=8, T=1024, D=256, NH=4, N=256, NLAYER=3.
Sharding: data-parallel over batch B — one batch element per NeuronCore (8 cores).

Algorithmic structure per core (b fixed), per layer:
  - fp16 matmul operands everywhere (f32 PSUM accumulation), f32 elementwise;
    x kept in three layouts: xf (f32 [t,d] tiles), xb (fp16 [t,d] + ones
    column, attn rhs), xT (fp16 [d,t], encoder rhs, via DMA transposes).
  - encoder matmul run twice (normal + column-pair-swapped weights) so RoPE in
    the [n,t] layout is 3 elementwise ops against precomputed cos/sin tables
    (relu fused into the first two via scalar_tensor_tensor reading PSUM).
  - scores = QR^T QR is symmetric, so exp(scores*scale) tiles serve as both
    lhsT and rhs without transposes; softmax max-subtraction is skipped
    (|scores*scale| is bounded ~1) and normalization is deferred: row-sums come
    free from the Exp activation's accum_out, and 1/rowsum is folded into the
    following LayerNorm as denom = sqrt(var_u + eps*rowsum^2).
  - yKV LN stats via bn_stats/bn_aggr on the f32 PSUM tile; mean-subtraction
    is skipped where the input has exact zero row-mean by construction; softmax
    row-sums come free as a ones-column of the attn matmul rhs;
    1/sqrt(var+eps') computed as Exp(-0.5*Ln(.)) so every ACT function used
    (Exp/Ln/Relu/Copy/Identity) lives in one activation-table set (no table
    reload thrash; see _patch_act_tables).
  - gate xy = relu(encv-matmul) * x_sparse fused into one DVE op per tile.
  - decoder matmul consumes the gated tiles as lhsT directly; double LN with
    residual; final logits via tensor_tensor_reduce (row-dot with out_w).
"""

import sys

sys.path.insert(0, "/opt/trn_rl_repo")

import numpy as np

B, T, D, NH, NLAYER = 8, 1024, 256, 4, 3
N = 256
GRID = 32
EPS = 1e-5
SCALE = 1.0 / 16.0  # 1/sqrt(N)
P = 128

_CACHE = {}


def _rope_tables():
    """cos/sin tables in [n, t] layout (f32 [256, 1024]), sin pre-signed."""
    dim_half = N // 2  # 128
    freqs = 1.0 / (
        10000.0 ** (np.arange(0, dim_half, 2, dtype=np.float32) / dim_half)
    )  # [64], float64 like the reference
    fx = np.outer(np.arange(GRID), freqs)  # [32, 64] angle for w coord
    fy = np.outer(np.arange(GRID), freqs)  # [32, 64] angle for h coord
    ww = np.tile(np.arange(GRID), GRID)  # t -> w = t % 32
    hh = np.repeat(np.arange(GRID), GRID)  # t -> h = t // 32
    ang = np.concatenate([fx[ww], fy[hh]], axis=1)  # [1024, 128]
    C = np.cos(ang).astype(np.float32)  # [T, 128]
    S = np.sin(ang).astype(np.float32)
    cosT = np.repeat(C.T, 2, axis=0)  # [256, 1024]
    sinT = np.repeat(S.T, 2, axis=0)
    sinT[0::2, :] *= -1.0  # even n rows: -sin
    return np.ascontiguousarray(cosT), np.ascontiguousarray(sinT)


def _emit(nc, tc, ap):
    from contextlib import ExitStack

    from concourse import mybir
    F32 = mybir.dt.float32
    BF16 = mybir.dt.float16
    Alu = mybir.AluOpType
    ACTF = mybir.ActivationFunctionType
    AXX = mybir.AxisListType.X

    import os as _os2

    ctx = ExitStack()
    const = ctx.enter_context(tc.tile_pool(name="const", bufs=1))
    state = ctx.enter_context(tc.tile_pool(name="state", bufs=int(_os2.environ.get("BDH_STATE", "2"))))
    work = ctx.enter_context(tc.tile_pool(name="work", bufs=int(_os2.environ.get("BDH_WORK", "3"))))
    epool = ctx.enter_context(tc.tile_pool(name="epool", bufs=int(_os2.environ.get("BDH_EPOOL", "3"))))
    xypool = ctx.enter_context(tc.tile_pool(name="xy", bufs=1))
    stat = ctx.enter_context(tc.tile_pool(name="stat", bufs=int(_os2.environ.get("BDH_STAT", "6"))))
    _pb = [int(x) for x in _os2.environ.get("BDH_PSUM", "3,2,2,1").split(",")]
    psA = ctx.enter_context(tc.tile_pool(name="psA", bufs=_pb[0], space="PSUM"))
    psS = ctx.enter_context(tc.tile_pool(name="psS", bufs=_pb[1], space="PSUM"))
    psY = ctx.enter_context(tc.tile_pool(name="psY", bufs=_pb[2], space="PSUM"))
    psH = ctx.enter_context(tc.tile_pool(name="psH", bufs=_pb[3], space="PSUM"))

    # ---- constants / weights to SBUF ----
    def bcast(src_ap, parts):
        import concourse.bass as bass

        return bass.AP(
            tensor=src_ap.tensor,
            offset=src_ap.offset,
            ap=[[0, parts]] + [list(x) for x in src_ap.ap],
        )

    winb = const.tile([P, D], F32, tag="winb", name="winb")
    nc.gpsimd.dma_start(out=winb, in_=bcast(ap["inw"][0, :], P))
    binb = const.tile([P, D], F32, tag="binb", name="binb")
    nc.gpsimd.dma_start(out=binb, in_=bcast(ap["inb"][0, :], P))
    woutb = const.tile([P, D], F32, tag="woutb", name="woutb")
    nc.gpsimd.dma_start(out=woutb, in_=bcast(ap["outw"][0, :], P))
    outbb = const.tile([P, 1], F32, tag="outbb", name="outbb")
    nc.gpsimd.dma_start(out=outbb, in_=bcast(ap["outb"][0, :], P))
    epsc = const.tile([P, 1], F32, tag="epsc", name="epsc")
    nc.vector.memset(epsc, float(EPS))

    encS = [[None] * 2 for _ in range(NH)]
    encswS = [[None] * 2 for _ in range(NH)]
    encvS = [[None] * 2 for _ in range(NH)]
    for h in range(NH):
        for k in range(2):
            for nm, dst, src in (
                ("enc", encS, ap["enc"]),
                ("encsw", encswS, ap["encsw"]),
                ("encv", encvS, ap["encv"]),
            ):
                t = const.tile([P, N], BF16, tag=f"{nm}{h}{k}", name=f"{nm}{h}{k}")
                eng = nc.sync if (h + k) % 2 == 0 else nc.gpsimd
                eng.dma_start(out=t, in_=src[h, k * P : (k + 1) * P, :])
                dst[h][k] = t
    decS = []
    for k in range(8):
        t = const.tile([P, D], BF16, tag=f"dec{k}", name=f"dec{k}")
        eng = nc.sync if k % 2 == 0 else nc.gpsimd
        eng.dma_start(out=t, in_=ap["dec"][k * P : (k + 1) * P, :])
        decS.append(t)
    cosS = []
    sinS = []
    for nt in range(2):
        t = const.tile([P, T], BF16, tag=f"cos{nt}", name=f"cos{nt}")
        nc.sync.dma_start(out=t, in_=ap["cost"][nt * P : (nt + 1) * P, :])
        cosS.append(t)
        t = const.tile([P, T], BF16, tag=f"sin{nt}", name=f"sin{nt}")
        nc.gpsimd.dma_start(out=t, in_=ap["sint"][nt * P : (nt + 1) * P, :])
        sinS.append(t)
    # ---- helpers ----
    def ln_stats(src_ap, eps_tile=None, skip_mean=False):
        """den = 1/sqrt(var + eps') where eps' is EPS or a precomputed
        per-partition tile (eps*rowsum^2, deferred-softmax fold). When
        skip_mean (input rows have exact zero mean by construction), negmd is
        omitted."""
        st = stat.tile([P, 6], F32, tag="st", name="st")
        nc.vector.bn_stats(out=st, in_=src_ap)
        mv = stat.tile([P, 2], F32, tag="mv", name="mv")
        nc.vector.bn_aggr(out=mv, in_=st)
        q = stat.tile([P, 1], F32, tag="q", name="q")
        nc.scalar.activation(
            out=q,
            in_=mv[:, 1:2],
            func=ACTF.Ln,
            bias=eps_tile if eps_tile is not None else epsc,
            scale=1.0,
        )
        den = stat.tile([P, 1], F32, tag="den", name="den")
        nc.scalar.activation(out=den, in_=q, func=ACTF.Exp, scale=-0.5)
        if skip_mean:
            return den, None
        negmd = stat.tile([P, 1], F32, tag="negmd", name="negmd")
        nc.vector.scalar_tensor_tensor(
            out=negmd, in0=mv[:, 0:1], scalar=-1.0, in1=den, op0=Alu.mult, op1=Alu.mult
        )
        return den, negmd

    def finish_x(p, src_ap, den, negmd, xT_new, need_next):
        """Evict normalized x tile (f32 + bf16) and write xT slices."""
        nxf = state.tile([P, D], F32, tag=f"xf{p}", name=f"xf{p}")
        if negmd is None:
            nc.scalar.activation(out=nxf, in_=src_ap, func=ACTF.Copy, scale=den)
        else:
            nc.scalar.activation(
                out=nxf, in_=src_ap, func=ACTF.Identity, scale=den, bias=negmd
            )
        if not need_next:
            return nxf, None
        nxb = state.tile([P, D + 1], BF16, tag=f"xb{p}", name=f"xb{p}")
        nc.vector.tensor_copy(out=nxb[:, 0:D], in_=nxf)
        nc.gpsimd.memset(nxb[:, D : D + 1], 1.0)
        for dt in range(2):
            nc.sync.dma_start(
                out=xT_new[:, dt, p * P : (p + 1) * P],
                in_=nxb[:, dt * P : (dt + 1) * P],
                transpose=True,
            )
        return nxf, nxb

    import os as _os
    _reps = int(_os.environ.get("BDH_REPS", "0") or "0") or getattr(tc, "_bdh_reps", 1)
    for _rep in range(_reps):
        # ---- input projection + LN -> x0 ----
        xf = [None] * 8
        xb = [None] * 8
        xT = state.tile([P, 2, T], BF16, tag="xT", name="xTi")
        for p in range(8):
            uc = stat.tile([P, 1], F32, tag="uc", name="uc")
            nc.sync.dma_start(out=uc, in_=ap["u"][p * P : (p + 1) * P, :])
            t0 = work.tile([P, D], F32, tag="t0", name="t0", bufs=3)
            nc.vector.tensor_scalar(
                out=t0, in0=winb, scalar1=uc, scalar2=None, op0=Alu.mult
            )
            nc.vector.tensor_tensor(out=t0, in0=t0, in1=binb, op=Alu.add)
            den, negmd = ln_stats(t0)
            xf[p], xb[p] = finish_x(p, t0, den, negmd, xT, True)

        # ---- layers ----
        for L in range(NLAYER):
            last = L == NLAYER - 1
            xyT = [
                xypool.tile([P, T], BF16, tag=f"xyT{k}", name=f"xyT{k}_{L}")
                for k in range(8)
            ]
            for h in range(NH):
                # A/B: encoder matmuls (normal + swapped) + relu + rope
                xsT = [
                    work.tile([P, T], F32, tag=f"xsT{nt}", name=f"xsT{nt}_{L}{h}")
                    for nt in range(2)
                ]
                qrt = [
                    work.tile([P, T], BF16, tag=f"qrt{nt}", name=f"qrt{nt}_{L}{h}")
                    for nt in range(2)
                ]
                for nt in range(2):
                    t1 = work.tile([P, T], BF16, tag="t1", name=f"t1_{L}{h}{nt}")
                    t2 = work.tile([P, T], BF16, tag="t2", name=f"t2_{L}{h}{nt}")
                    for tch in range(2):
                        sl = slice(tch * 512, (tch + 1) * 512)
                        pre = psA.tile([P, 512], F32, tag="pre", name=f"pre{L}{h}{nt}{tch}")
                        for kt in range(2):
                            nc.tensor.matmul(
                                pre,
                                encS[h][kt][:, nt * P : (nt + 1) * P],
                                xT[:, kt, sl],
                                start=(kt == 0),
                                stop=(kt == 1),
                            )
                        presw = psA.tile([P, 512], F32, tag="pre", name=f"presw{L}{h}{nt}{tch}")
                        for kt in range(2):
                            nc.tensor.matmul(
                                presw,
                                encswS[h][kt][:, nt * P : (nt + 1) * P],
                                xT[:, kt, sl],
                                start=(kt == 0),
                                stop=(kt == 1),
                            )
                        nc.scalar.activation(out=xsT[nt][:, sl], in_=pre, func=ACTF.Relu)
                        nc.vector.scalar_tensor_tensor(
                            out=t1[:, sl],
                            in0=pre,
                            scalar=0.0,
                            in1=cosS[nt][:, sl],
                            op0=Alu.max,
                            op1=Alu.mult,
                        )
                        nc.vector.scalar_tensor_tensor(
                            out=t2[:, sl],
                            in0=presw,
                            scalar=0.0,
                            in1=sinS[nt][:, sl],
                            op0=Alu.max,
                            op1=Alu.mult,
                        )
                    nc.vector.tensor_tensor(out=qrt[nt], in0=t1, in1=t2, op=Alu.add)

                # D: scores + exp (+ row sums)
                E = [
                    epool.tile([P, T], BF16, tag=f"E{p}", name=f"E{p}_{L}{h}")
                    for p in range(8)
                ]
                for p in range(8):
                    for ch in range(2):
                        ps = psS.tile([P, 512], F32, tag="s", name=f"ps{L}{h}{p}{ch}")
                        for kt in range(2):
                            nc.tensor.matmul(
                                ps,
                                qrt[kt][:, p * P : (p + 1) * P],
                                qrt[kt][:, ch * 512 : (ch + 1) * 512],
                                start=(kt == 0),
                                stop=(kt == 1),
                            )
                        nc.scalar.activation(
                            out=E[p][:, ch * 512 : (ch + 1) * 512],
                            in_=ps,
                            func=ACTF.Exp,
                            scale=SCALE,
                        )

                # E-stage: yKV = E @ x (unnormalized), folded LN, transpose
                ylnT = work.tile([P, 2, T], BF16, tag="ylnT", name=f"ylnT_{L}{h}")
                for p in range(8):
                    py = psY.tile([P, D + 1], F32, tag="y", name=f"py{L}{h}{p}")
                    for s in range(8):
                        nc.tensor.matmul(
                            py,
                            E[s][:, p * P : (p + 1) * P],
                            xb[s],
                            start=(s == 0),
                            stop=(s == 7),
                        )
                    rss = stat.tile([P, 1], F32, tag="rss", name=f"rss{L}{h}{p}")
                    nc.vector.tensor_scalar(
                        out=rss,
                        in0=py[:, D : D + 1],
                        scalar1=float(EPS) ** 0.5,
                        scalar2=None,
                        op0=Alu.mult,
                    )
                    rs2e = stat.tile([P, 1], F32, tag="rs2e", name=f"rs2e{L}{h}{p}")
                    nc.vector.tensor_tensor(out=rs2e, in0=rss, in1=rss, op=Alu.mult)
                    den, _ = ln_stats(py[:, 0:D], eps_tile=rs2e, skip_mean=True)
                    yln = work.tile([P, D], BF16, tag="yln", name=f"yln{L}{h}{p}", bufs=3)
                    nc.vector.tensor_scalar(
                        out=yln, in0=py[:, 0:D], scalar1=den, scalar2=None, op0=Alu.mult,
                    )
                    for dt in range(2):
                        nc.sync.dma_start(
                            out=ylnT[:, dt, p * P : (p + 1) * P],
                            in_=yln[:, dt * P : (dt + 1) * P],
                            transpose=True,
                        )

                # H: encoder_v matmul + fused relu*xs gate
                for nt in range(2):
                    for tch in range(2):
                        sl = slice(tch * 512, (tch + 1) * 512)
                        pyv = psH.tile([P, 512], F32, tag="h", name=f"pyv{L}{h}{nt}{tch}")
                        for kt in range(2):
                            nc.tensor.matmul(
                                pyv,
                                encvS[h][kt][:, nt * P : (nt + 1) * P],
                                ylnT[:, kt, sl],
                                start=(kt == 0),
                                stop=(kt == 1),
                            )
                        nc.vector.scalar_tensor_tensor(
                            out=xyT[h * 2 + nt][:, sl],
                            in0=pyv,
                            scalar=0.0,
                            in1=xsT[nt][:, sl],
                            op0=Alu.max,
                            op1=Alu.mult,
                        )

            # J: decoder matmul + LN(x + LN(yMLP))
            new_xT = (
                None
                if last
                else state.tile([P, 2, T], BF16, tag="xT", name=f"xT_{L}")
            )
            new_xf = [None] * 8
            new_xb = [None] * 8
            for p in range(8):
                pm = psY.tile([P, D], F32, tag="y", name=f"pm{L}{p}")
                for k in range(8):
                    nc.tensor.matmul(
                        pm,
                        xyT[k][:, p * P : (p + 1) * P],
                        decS[k],
                        start=(k == 0),
                        stop=(k == 7),
                    )
                den1, negmd1 = ln_stats(pm)
                ln1 = work.tile([P, D], F32, tag="ln1", name=f"ln1_{L}{p}", bufs=3)
                nc.scalar.activation(
                    out=ln1, in_=pm, func=ACTF.Identity, scale=den1, bias=negmd1
                )
                z = work.tile([P, D], F32, tag="z", name=f"z{L}{p}", bufs=3)
                nc.vector.tensor_tensor(out=z, in0=xf[p], in1=ln1, op=Alu.add)
                den2, negmd2 = ln_stats(z)
                new_xf[p], new_xb[p] = finish_x(p, z, den2, negmd2, new_xT, not last)
            xf, xb, xT = new_xf, new_xb, new_xT

        # ---- logits ----
        for p in range(8):
            tmp = work.tile([P, D], F32, tag="lgt", name=f"lgt{p}")
            lg = stat.tile([P, 1], F32, tag="lg", name=f"lg{p}")
            nc.vector.tensor_tensor(out=tmp, in0=xf[p], in1=woutb, op=Alu.mult)
            nc.vector.reduce_sum(out=lg, in_=tmp, axis=AXX)
            nc.vector.tensor_scalar(
                out=lg, in0=lg, scalar1=outbb, scalar2=None, op0=Alu.add
            )
            nc.sync.dma_start(out=ap["y"][p * P : (p + 1) * P, :], in_=lg)


    ctx.close()


def _patch_act_tables():
    """All ACT funcs used here (Exp, Ln, Relu, Copy, Identity) live in the
    natural_log_exp_and_others set; empty the others so the table-load pass
    settles on one set and elides every reload (keeps act_func_set ids)."""
    if _CACHE.get("act_patched"):
        return
    import concourse.bacc as bacc
    import concourse.bass_interp as bass_interp

    KEEP = "natural_log_exp_and_others"

    def filtered(orig):
        def f(arch):
            t = orig(arch)
            return {k: (v if k == KEEP else set()) for k, v in t.items()}

        return f

    bacc.get_activation_tables = filtered(bacc.get_activation_tables)
    bass_interp.get_activation_tables = filtered(bass_interp.get_activation_tables)
    _CACHE["act_patched"] = True


def _build(reps=1):
    import concourse.bacc as bacc
    import concourse.tile as tile
    from concourse import mybir

    _patch_act_tables()

    F32 = mybir.dt.float32
    BF16 = mybir.dt.float16

    nc = bacc.Bacc(
        "TRN2",
        target_bir_lowering=False,
        debug=False,
        enable_asserts=True,
        num_devices=8,
    )
    ap = {}
    specs = [
        ("u", [T, 1], F32),
        ("inw", [1, D], F32),
        ("inb", [1, D], F32),
        ("enc", [NH, D, N], BF16),
        ("encsw", [NH, D, N], BF16),
        ("encv", [NH, D, N], BF16),
        ("dec", [NH * N, D], BF16),
        ("cost", [N, T], BF16),
        ("sint", [N, T], BF16),
        ("outw", [1, D], F32),
        ("outb", [1, 1], F32),
    ]
    for name, shape, dt in specs:
        ap[name] = nc.dram_tensor(name, shape, dt, kind="ExternalInput").ap()
    ap["y"] = nc.dram_tensor("y", [T, 1], F32, kind="ExternalOutput").ap()

    with tile.TileContext(nc) as tc:
        tc._bdh_reps = reps
        _emit(nc, tc, ap)
    nc.compile()
    return nc


def get_nc(reps=1):
    key = f"nc{reps}"
    if key not in _CACHE:
        _CACHE[key] = _build(reps)
    return _CACHE[key]


def make_in_maps(inputs, in_w, in_b, encoder, encoder_v, decoder, out_w, out_b):
    import ml_dtypes

    bf = np.float16
    cosT, sinT = _rope_tables()
    swap = np.arange(N) ^ 1
    common = {
        "inw": np.ascontiguousarray(in_w.reshape(1, D)).astype(np.float32),
        "inb": np.ascontiguousarray(in_b.reshape(1, D)).astype(np.float32),
        "enc": np.ascontiguousarray(encoder).astype(bf),
        "encsw": np.ascontiguousarray(encoder[:, :, swap]).astype(bf),
        "encv": np.ascontiguousarray(encoder_v).astype(bf),
        "dec": np.ascontiguousarray(decoder).astype(bf),
        "cost": cosT.astype(bf),
        "sint": sinT.astype(bf),
        "outw": np.ascontiguousarray(out_w.reshape(1, D)).astype(np.float32),
        "outb": np.ascontiguousarray(out_b.reshape(1, 1)).astype(np.float32),
    }
    return [
        {"u": np.ascontiguousarray(inputs[b].reshape(T, 1)).astype(np.float32), **common}
        for b in range(B)
    ]


def get_runner(reps=1):
    """Cached jitted shard_map runner over 8 cores (mirrors
    bass2jax.run_bass_via_pjrt's multi-core path, but reusable across calls)."""
    key = f"runner{reps}"
    if key in _CACHE:
        return _CACHE[key]
    import jax
    from jax.experimental.shard_map import shard_map
    from jax.sharding import Mesh, PartitionSpec

    from concourse import mybir
    from concourse.bass2jax import (
        _bass_exec_p,
        install_neuronx_cc_hook,
        partition_id_tensor,
    )

    nc = get_nc(reps)
    install_neuronx_cc_hook()

    partition_name = nc.partition_id_tensor.name if nc.partition_id_tensor else None
    in_names, out_names, out_avals, zero_outs = [], [], [], []
    for alloc in nc.m.functions[0].allocations:
        if not isinstance(alloc, mybir.MemoryLocationSet):
            continue
        name = alloc.memorylocations[0].name
        if alloc.kind == "ExternalInput":
            if name != partition_name:
                in_names.append(name)
        elif alloc.kind == "ExternalOutput":
            shape = tuple(alloc.tensor_shape)
            dtype = mybir.dt.np(alloc.dtype)
            out_names.append(name)
            out_avals.append(jax.core.ShapedArray(shape, dtype))
            zero_outs.append(np.zeros(shape, dtype))
    n_params = len(in_names)
    all_in_names = in_names + out_names
    if partition_name is not None:
        all_in_names = all_in_names + [partition_name]
    donate = tuple(range(n_params, n_params + len(out_names)))

    def _body(*args):
        operands = list(args)
        if partition_name is not None:
            operands.append(partition_id_tensor())
        outs = _bass_exec_p.bind(
            *operands,
            out_avals=tuple(out_avals),
            in_names=tuple(all_in_names),
            out_names=tuple(out_names),
            lowering_input_output_aliases=(),
            sim_require_finite=True,
            sim_require_nnan=True,
            nc=nc,
        )
        return tuple(outs)

    devices = jax.devices()[:B]
    mesh = Mesh(np.asarray(devices), ("core",))
    in_specs = (PartitionSpec("core"),) * (n_params + len(out_names))
    out_specs = (PartitionSpec("core"),) * len(out_names)
    sharded = jax.jit(
        shard_map(
            _body, mesh=mesh, in_specs=in_specs, out_specs=out_specs, check_rep=False
        ),
        donate_argnums=donate,
        keep_unused=True,
    )

    runner = {
        "sharded": sharded,
        "in_names": in_names,
        "out_names": out_names,
        "zero_outs": zero_outs,
        "n_params": n_params,
        "mesh": mesh,
    }
    _CACHE[key] = runner
    return runner


def run_on_device(in_maps, iters=1):
    """Run the kernel `iters` times; returns (list of per-core out dicts,
    per-iteration wall seconds over the last iters-1 runs or the single run)."""
    import time

    import jax

    r = get_runner()
    concat_in = [
        np.concatenate([np.asarray(m[name]) for m in in_maps], axis=0)
        for name in r["in_names"]
    ]
    concat_in = [jax.device_put(a) for a in concat_in]
    for a in concat_in:
        a.block_until_ready()

    def one_call():
        zeros = [
            np.zeros((B * z.shape[0], *z.shape[1:]), z.dtype) for z in r["zero_outs"]
        ]
        return r["sharded"](*concat_in, *zeros)

    outs = one_call()  # compile + first run
    for o in outs:
        o.block_until_ready()
    per_iter = None
    if iters > 1:
        t0 = time.perf_counter()
        for _ in range(iters - 1):
            outs = one_call()
        for o in outs:
            o.block_until_ready()
        per_iter = (time.perf_counter() - t0) / (iters - 1)
    results = []
    for c in range(B):
        d = {}
        for i, name in enumerate(r["out_names"]):
            full = np.asarray(outs[i])
            pershape = r["zero_outs"][i].shape
            d[name] = full.reshape(B, *pershape)[c]
        results.append(d)
    return results, per_iter


def bench_chain(in_maps, k=20):
    """Run the kernel k times inside ONE jitted call, chaining y -> u to force
    sequential execution; returns per-iteration seconds (amortizes dispatch)."""
    import time

    import jax
    import jax.numpy as jnp
    from jax.experimental.shard_map import shard_map
    from jax.sharding import Mesh, PartitionSpec

    from concourse import mybir
    from concourse.bass2jax import (
        _bass_exec_p,
        install_neuronx_cc_hook,
        partition_id_tensor,
    )

    nc = get_nc(reps)
    install_neuronx_cc_hook()
    partition_name = nc.partition_id_tensor.name if nc.partition_id_tensor else None
    in_names, out_names, out_avals, zero_outs = [], [], [], []
    for alloc in nc.m.functions[0].allocations:
        if not isinstance(alloc, mybir.MemoryLocationSet):
            continue
        name = alloc.memorylocations[0].name
        if alloc.kind == "ExternalInput":
            if name != partition_name:
                in_names.append(name)
        elif alloc.kind == "ExternalOutput":
            shape = tuple(alloc.tensor_shape)
            dtype = mybir.dt.np(alloc.dtype)
            out_names.append(name)
            out_avals.append(jax.core.ShapedArray(shape, dtype))
            zero_outs.append(np.zeros(shape, dtype))
    all_in_names = in_names + out_names
    if partition_name is not None:
        all_in_names = all_in_names + [partition_name]
    ui = in_names.index("u")
    yi = out_names.index("y")

    def _one(args):
        operands = list(args) + [jnp.zeros_like(jnp.asarray(z)) for z in zero_outs]
        if partition_name is not None:
            operands.append(partition_id_tensor())
        return _bass_exec_p.bind(
            *operands,
            out_avals=tuple(out_avals),
            in_names=tuple(all_in_names),
            out_names=tuple(out_names),
            lowering_input_output_aliases=(),
            sim_require_finite=True,
            sim_require_nnan=True,
            nc=nc,
        )

    def _chain(*args):
        import jax.lax as lax

        args = list(args)

        def step(u, _):
            a = list(args)
            a[ui] = u
            outs = _one(a)
            y = outs[yi]
            return u + 0.0 * y, ()

        u_fin, _ = lax.scan(step, args[ui], None, length=k)
        return u_fin

    devices = jax.devices()[:B]
    mesh = Mesh(np.asarray(devices), ("core",))
    in_specs = (PartitionSpec("core"),) * len(in_names)
    out_specs = PartitionSpec("core")
    chained = jax.jit(
        shard_map(
            _chain, mesh=mesh, in_specs=in_specs, out_specs=out_specs, check_rep=False
        )
    )
    concat_in = [
        np.concatenate([np.asarray(m[name]) for m in in_maps], axis=0)
        for name in in_names
    ]
    concat_in = [jax.device_put(a) for a in concat_in]
    for a in concat_in:
        a.block_until_ready()
    out = chained(*concat_in)
    out.block_until_ready()  # compile + warm
    t0 = time.perf_counter()
    out = chained(*concat_in)
    out.block_until_ready()
    t1 = time.perf_counter()
    return (t1 - t0) / k, t1 - t0


def kernel(inputs, in_w, in_b, encoder, encoder_v, decoder, out_w, out_b):
    inputs = np.asarray(inputs)
    in_maps = make_in_maps(
        np.asarray(inputs, np.float32),
        np.asarray(in_w, np.float32),
        np.asarray(in_b, np.float32),
        np.asarray(encoder, np.float32),
        np.asarray(encoder_v, np.float32),
        np.asarray(decoder, np.float32),
        np.asarray(out_w, np.float32),
        np.asarray(out_b, np.float32),
    )
    results, _ = run_on_device(in_maps, iters=1)
    out = np.stack([results[b]["y"] for b in range(B)], axis=0)  # (8, 1024, 1)
    return out.astype(np.float32)


if __name__ == "__main__":
    rng = np.random.default_rng(0)
    out = kernel(
        inputs=rng.standard_normal((B, T), dtype=np.float32),
        in_w=rng.standard_normal((D, 1), dtype=np.float32) * 0.02,
        in_b=np.zeros((D,), np.float32),
        encoder=rng.standard_normal((NH, D, N), dtype=np.float32) * 0.02,
        encoder_v=rng.standard_normal((NH, D, N), dtype=np.float32) * 0.02,
        decoder=rng.standard_normal((NH * N, D), dtype=np.float32) * 0.02,
        out_w=rng.standard_normal((1, D), dtype=np.float32) * 0.02,
        out_b=np.zeros((1,), np.float32),
    )
    print("out", out.shape, out.dtype, np.abs(out).max())



# revision 28
# speedup vs baseline: 1.0877x; 1.0877x over previous
"""Trainium2 Bass kernel for the BDH dense-transformer problem.

Shapes (hardcoded): B=8, T=1024, D=256, NH=4, N=256, NLAYER=3.
Sharding: data-parallel over batch B — one batch element per NeuronCore (8 cores).

Algorithmic structure per core (b fixed), per layer:
  - fp16 matmul operands (f32 PSUM accumulation); x kept in three layouts:
    xf (f32 [t,d] tiles), xb (fp16 [t,d] + ones column, attn rhs), xT (fp16
    [d,t], encoder rhs, via merged 3D-out DMA transposes: one DmaTransposeAnt
    per 128-row chunk writes both 128-partition d-chunks).
  - RoPE in the [n,t] layout with all-SBUF fp16 scalar_tensor_tensor ops
    (4x DVE mode): relu lands x_sparse in fp16 SBUF (ACT), then
    qr = xs*cos + swap(xs)*sin where the pair-swap is two partition-stride-2
    ops; no second (column-swapped) encoder matmul needed.
  - scores = QR^T QR is symmetric, so exp(scores*scale) tiles serve as both
    lhsT and rhs without transposes; softmax max-subtraction is skipped
    (|scores*scale| is bounded ~1) and normalization is deferred: row-sums come
    free as a ones-column of the attn matmul rhs, and 1/rowsum is folded into
    the following LayerNorm as denom = sqrt(var_u + eps*rowsum^2).
  - yKV LN stats via bn_stats/bn_aggr on the f32 PSUM tile; mean-subtraction
    skipped (x rows have exact zero mean); 1/sqrt(var+eps') computed as
    Exp(-0.5*Ln(.)) so every ACT function used lives in one activation-table
    set (no table reload thrash; see _patch_act_tables).
  - gate xy = relu(encv-matmul) * x_sparse fused into one DVE op per tile.
  - decoder matmul consumes the gated tiles as lhsT directly; double LN with
    residual (LN1 fused into one two-scalar tensor_scalar op; residual add and
    fp16 recast offloaded to the idle Pool/gpsimd engine); final logits via
    row-dot with out_w.
"""

import sys

sys.path.insert(0, "/opt/trn_rl_repo")

import numpy as np

B, T, D, NH, NLAYER = 8, 1024, 256, 4, 3
N = 256
GRID = 32
EPS = 1e-5
SCALE = 1.0 / 16.0  # 1/sqrt(N)
P = 128

_CACHE = {}


def _rope_tables():
    """cos/sin tables in [n, t] layout (f32 [256, 1024]), sin pre-signed."""
    dim_half = N // 2  # 128
    freqs = 1.0 / (
        10000.0 ** (np.arange(0, dim_half, 2, dtype=np.float32) / dim_half)
    )  # [64], float64 like the reference
    fx = np.outer(np.arange(GRID), freqs)  # [32, 64] angle for w coord
    fy = np.outer(np.arange(GRID), freqs)  # [32, 64] angle for h coord
    ww = np.tile(np.arange(GRID), GRID)  # t -> w = t % 32
    hh = np.repeat(np.arange(GRID), GRID)  # t -> h = t // 32
    ang = np.concatenate([fx[ww], fy[hh]], axis=1)  # [1024, 128]
    C = np.cos(ang).astype(np.float32)  # [T, 128]
    S = np.sin(ang).astype(np.float32)
    cosT = np.repeat(C.T, 2, axis=0)  # [256, 1024]
    sinT = np.repeat(S.T, 2, axis=0)
    sinT[0::2, :] *= -1.0  # even n rows: -sin
    return np.ascontiguousarray(cosT), np.ascontiguousarray(sinT)


def _emit(nc, tc, ap):
    from contextlib import ExitStack

    from concourse import mybir
    F32 = mybir.dt.float32
    BF16 = mybir.dt.float16
    Alu = mybir.AluOpType
    ACTF = mybir.ActivationFunctionType
    AXX = mybir.AxisListType.X

    import os as _os2

    ctx = ExitStack()
    const = ctx.enter_context(tc.tile_pool(name="const", bufs=1))
    state = ctx.enter_context(tc.tile_pool(name="state", bufs=int(_os2.environ.get("BDH_STATE", "2"))))
    work = ctx.enter_context(tc.tile_pool(name="work", bufs=int(_os2.environ.get("BDH_WORK", "2"))))
    epool = ctx.enter_context(tc.tile_pool(name="epool", bufs=int(_os2.environ.get("BDH_EPOOL", "2"))))
    xypool = ctx.enter_context(tc.tile_pool(name="xy", bufs=int(_os2.environ.get("BDH_XY", "1"))))
    stat = ctx.enter_context(tc.tile_pool(name="stat", bufs=int(_os2.environ.get("BDH_STAT", "6"))))
    _pb = [int(x) for x in _os2.environ.get("BDH_PSUM", "2,2,3,1").split(",")]
    psA = ctx.enter_context(tc.tile_pool(name="psA", bufs=_pb[0], space="PSUM"))
    psS = ctx.enter_context(tc.tile_pool(name="psS", bufs=_pb[1], space="PSUM"))
    psY = ctx.enter_context(tc.tile_pool(name="psY", bufs=_pb[2], space="PSUM"))
    psH = ctx.enter_context(tc.tile_pool(name="psH", bufs=_pb[3], space="PSUM"))

    # ---- constants / weights to SBUF ----
    def bcast(src_ap, parts):
        import concourse.bass as bass

        return bass.AP(
            tensor=src_ap.tensor,
            offset=src_ap.offset,
            ap=[[0, parts]] + [list(x) for x in src_ap.ap],
        )

    import concourse.bass as bass

    def dram_view(src_ap, dims):
        """Raw multi-dim view of a DRAM tensor: dims = [[stride, num], ...]."""
        return bass.AP(tensor=src_ap.tensor, offset=src_ap.offset, ap=dims)

    # Startup is HWDGE/SWDGE instruction-count bound (~625-1000ns per DMA
    # instruction), so weights are fetched with a handful of big strided
    # DMAs: packed scalars in one bcast load, enc/encv as [128, NH, N] per
    # d-chunk, dec as [128, 8, D], cos/sin as [128, 2, T].
    # consts layout (see make_in_maps): [in_w | in_b | out_w | out_b] (3D+1)
    cpack = const.tile([P, 3 * D + 1], F32, tag="cpack", name="cpack")
    nc.sync.dma_start(out=cpack, in_=bcast(ap["consts"][0, :], P))
    winb = cpack[:, 0:D]
    binb = cpack[:, D : 2 * D]
    woutb = cpack[:, 2 * D : 3 * D]
    outbb = cpack[:, 3 * D : 3 * D + 1]
    epsc = const.tile([P, 1], F32, tag="epsc", name="epsc")
    nc.vector.memset(epsc, float(EPS))

    # enc[h, k*128+d, n] -> encA[k][d, h, n]
    encA = []
    for k in range(2):
        t = const.tile([P, NH, N], BF16, tag=f"enc{k}", name=f"enc{k}")
        nc.sync.dma_start(
            out=t,
            in_=dram_view(ap["enc"][0, k * P, 0], [[N, P], [D * N, NH], [1, N]]),
        )
        encA.append(t)
    encS = [[encA[k][:, h, :] for k in range(2)] for h in range(NH)]
    cosS = []
    sinS = []
    for nm, lst in (("cost", cosS), ("sint", sinS)):
        t = const.tile([P, 2, T], BF16, tag=nm, name=nm)
        eng = nc.sync if nm == "cost" else nc.gpsimd
        eng.dma_start(
            out=t, in_=dram_view(ap[nm][0, 0], [[T, P], [P * T, 2], [1, T]])
        )
        lst.extend([t[:, 0, :], t[:, 1, :]])
    encvA = []
    for k in range(2):
        t = const.tile([P, NH, N], BF16, tag=f"encv{k}", name=f"encv{k}")
        nc.gpsimd.dma_start(
            out=t,
            in_=dram_view(ap["encv"][0, k * P, 0], [[N, P], [D * N, NH], [1, N]]),
        )
        encvA.append(t)
    encvS = [[encvA[k][:, h, :] for k in range(2)] for h in range(NH)]
    decA = const.tile([P, 8, D], BF16, tag="dec", name="dec")
    nc.gpsimd.dma_start(
        out=decA, in_=dram_view(ap["dec"][0, 0], [[D, P], [P * D, 8], [1, D]])
    )
    decS = [decA[:, k, :] for k in range(8)]
    pswap = const.tile([P, P], BF16, tag="psw", name="psw")
    nc.sync.dma_start(out=pswap, in_=ap["psw"][:, :])

    # ---- helpers ----
    def ln_stats(src_ap, eps_tile=None, skip_mean=False):
        """den = 1/sqrt(var + eps') where eps' is EPS or a precomputed
        per-partition tile (eps*rowsum^2, deferred-softmax fold). When
        skip_mean (input rows have exact zero mean by construction), negmd is
        omitted."""
        st = stat.tile([P, 6], F32, tag="st", name="st")
        nc.vector.bn_stats(out=st, in_=src_ap)
        mv = stat.tile([P, 2], F32, tag="mv", name="mv")
        nc.vector.bn_aggr(out=mv, in_=st)
        q = stat.tile([P, 1], F32, tag="q", name="q")
        nc.scalar.activation(
            out=q,
            in_=mv[:, 1:2],
            func=ACTF.Ln,
            bias=eps_tile if eps_tile is not None else epsc,
            scale=1.0,
        )
        den = stat.tile([P, 1], F32, tag="den", name="den")
        nc.scalar.activation(out=den, in_=q, func=ACTF.Exp, scale=-0.5)
        if skip_mean:
            return den, None
        negmd = stat.tile([P, 1], F32, tag="negmd", name="negmd")
        nc.vector.scalar_tensor_tensor(
            out=negmd, in0=mv[:, 0:1], scalar=-1.0, in1=den, op0=Alu.mult, op1=Alu.mult
        )
        return den, negmd

    def finish_x(p, src_ap, den, negmd, xT_new, need_next, fast=False):
        """Evict normalized x tile (f32 + fp16) and write xT chunk (merged
        transpose: one instruction covers both d-chunks). fast=True keeps the
        fp16 recast off the Pool queue (lower latency / Pool busy)."""
        nxf = state.tile([P, D], F32, tag=f"xf{p}", name=f"xf{p}")
        if negmd is None:
            nc.scalar.activation(out=nxf, in_=src_ap, func=ACTF.Copy, scale=den)
        else:
            nc.scalar.activation(
                out=nxf, in_=src_ap, func=ACTF.Identity, scale=den, bias=negmd
            )
        if not need_next:
            return nxf, None
        nxb = state.tile([P, D + 1], BF16, tag=f"xb{p}", name=f"xb{p}")
        eng = nc.vector if fast else nc.gpsimd
        eng.tensor_copy(out=nxb[:, 0:D], in_=nxf)
        eng.memset(nxb[:, D : D + 1], 1.0)
        nc.sync.dma_start(
            out=xT_new[:, :, p * P : (p + 1) * P],
            in_=nxb[:, 0:D],
            transpose=True,
        )
        return nxf, nxb

    import os as _os
    _reps = int(_os.environ.get("BDH_REPS", "0") or "0") or getattr(tc, "_bdh_reps", 1)
    for _rep in range(_reps):
        # ---- layers ----
        VAR = _os.environ.get("BDH_VARIANT", "v3")
        YLN_SPLIT = _os.environ.get("BDH_YLN", "p")  # d=all-DVE, p=parity, a=all-ACT
        QADD_POOL = _os.environ.get("BDH_QADD_POOL", "1") == "1"

        def emit_A(L, h, xsT, tchs=(0, 1), cur_xT=None):
            """encoder matmul + relu -> x_sparse fp16 [n,t] chunks."""
            src_xT = cur_xT if cur_xT is not None else xT
            for tch in tchs:
                sl = slice(tch * 512, (tch + 1) * 512)
                for nt in range(2):
                    pre = psA.tile([P, 512], F32, tag="pre", name=f"pre{L}{h}{nt}{tch}")
                    for kt in range(2):
                        nc.tensor.matmul(
                            pre,
                            encS[h][kt][:, nt * P : (nt + 1) * P],
                            src_xT[:, kt, sl],
                            start=(kt == 0),
                            stop=(kt == 1),
                        )
                    nc.scalar.activation(out=xsT[nt][:, sl], in_=pre, func=ACTF.Relu)

        def emit_R(L, h, xsT, qrt, nt, boundary=False):
            """rope: qr = xs*cos + swap(xs*sin'') where sin'' is the
            pair-swapped signed sin table and swap() is a PE matmul against a
            pair-swap permutation (partition-strided compute APs are illegal
            on HW). boundary=True keeps the add off the (slow) Pool engine
            when it gates the next layer's first scores."""
            t1 = work.tile([P, T], BF16, tag="t1", name=f"t1_{L}{h}{nt}")
            u = work.tile([P, T], BF16, tag="t2", name=f"u_{L}{h}{nt}")
            nc.vector.tensor_tensor(out=t1, in0=xsT[nt], in1=cosS[nt], op=Alu.mult)
            nc.vector.tensor_tensor(out=u, in0=xsT[nt], in1=sinS[nt], op=Alu.mult)
            for tch in range(2):
                sl = slice(tch * 512, (tch + 1) * 512)
                psB = psA.tile([P, 512], F32, tag="pre", name=f"psB{L}{h}{nt}{tch}")
                nc.tensor.matmul(psB, pswap, u[:, sl], start=True, stop=True)
                nc.vector.scalar_tensor_tensor(
                    out=qrt[nt][:, sl], in0=psB, scalar=0.0, in1=t1[:, sl],
                    op0=Alu.add, op1=Alu.add,
                )

        EXP1024 = _os.environ.get("BDH_EXP1024", "0") == "1"

        def emit_S(L, h, qrt, E):
            """scores + exp (+ deferred row sums)."""
            for p in range(8):
                if EXP1024:
                    ps2 = psS.tile([P, 2, 512], F32, tag="s", name=f"ps{L}{h}{p}")
                    for ch in range(2):
                        for kt in range(2):
                            nc.tensor.matmul(
                                ps2[:, ch, :],
                                qrt[kt][:, p * P : (p + 1) * P],
                                qrt[kt][:, ch * 512 : (ch + 1) * 512],
                                start=(kt == 0),
                                stop=(kt == 1),
                            )
                    nc.scalar.activation(
                        out=E[p], in_=ps2, func=ACTF.Exp, scale=SCALE
                    )
                    continue
                for ch in range(2):
                    ps = psS.tile([P, 512], F32, tag="s", name=f"ps{L}{h}{p}{ch}")
                    for kt in range(2):
                        nc.tensor.matmul(
                            ps,
                            qrt[kt][:, p * P : (p + 1) * P],
                            qrt[kt][:, ch * 512 : (ch + 1) * 512],
                            start=(kt == 0),
                            stop=(kt == 1),
                        )
                    nc.scalar.activation(
                        out=E[p][:, ch * 512 : (ch + 1) * 512],
                        in_=ps,
                        func=ACTF.Exp,
                        scale=SCALE,
                    )

        def emit_Yp(L, h, E, ylnT, p):
            """yKV = E @ x for one 128-row chunk: matmuls, folded LN stats,
            scale, merged transpose."""
            py = psY.tile([P, D + 1], F32, tag="y", name=f"py{L}{h}{p}")
            for s in range(8):
                nc.tensor.matmul(
                    py,
                    E[s][:, p * P : (p + 1) * P],
                    xb[s],
                    start=(s == 0),
                    stop=(s == 7),
                )
            rss = stat.tile([P, 1], F32, tag="rss", name=f"rss{L}{h}{p}")
            nc.vector.tensor_scalar(
                out=rss, in0=py[:, D : D + 1], scalar1=float(EPS) ** 0.5,
                scalar2=None, op0=Alu.mult,
            )
            rs2e = stat.tile([P, 1], F32, tag="rs2e", name=f"rs2e{L}{h}{p}")
            nc.vector.tensor_tensor(out=rs2e, in0=rss, in1=rss, op=Alu.mult)
            den, _ = ln_stats(py[:, 0:D], eps_tile=rs2e, skip_mean=True)
            yln = work.tile([P, D], BF16, tag="yln", name=f"yln{L}{h}{p}", bufs=3)
            on_act = YLN_SPLIT == "a" or (YLN_SPLIT == "p" and p % 2 == 0)
            if on_act:
                nc.scalar.activation(
                    out=yln, in_=py[:, 0:D], func=ACTF.Copy, scale=den
                )
            else:
                nc.vector.tensor_scalar(
                    out=yln, in0=py[:, 0:D], scalar1=den, scalar2=None, op0=Alu.mult,
                )
            nc.sync.dma_start(
                out=ylnT[:, :, p * P : (p + 1) * P],
                in_=yln,
                transpose=True,
            )

        def emit_V(L, h, ylnT, xsT, xyT, tch):
            """encoder_v matmul + fused relu*xs gate for one 512-col chunk."""
            sl = slice(tch * 512, (tch + 1) * 512)
            for nt in range(2):
                pyv = psH.tile([P, 512], F32, tag="h", name=f"pyv{L}{h}{nt}{tch}")
                for kt in range(2):
                    nc.tensor.matmul(
                        pyv,
                        encvS[h][kt][:, nt * P : (nt + 1) * P],
                        ylnT[:, kt, sl],
                        start=(kt == 0),
                        stop=(kt == 1),
                    )
                nc.vector.scalar_tensor_tensor(
                    out=xyT[h * 2 + nt][:, sl],
                    in0=pyv,
                    scalar=0.0,
                    in1=xsT[nt][:, sl],
                    op0=Alu.max,
                    op1=Alu.mult,
                )

        def new_xsT(L, h):
            return [
                work.tile([P, T], BF16, tag=f"xsT{nt}", name=f"xsT{nt}_{L}{h}")
                for nt in range(2)
            ]

        def new_qrt(L, h):
            return [
                work.tile([P, T], BF16, tag=f"qrt{nt}", name=f"qrt{nt}_{L}{h}")
                for nt in range(2)
            ]

        def new_E(L, h):
            return [
                epool.tile([P, T], BF16, tag=f"E{p}", name=f"E{p}_{L}{h}")
                for p in range(8)
            ]

        # ---- input projection + LN -> x0 ----
        xf = [None] * 8
        xb = [None] * 8
        xT = state.tile([P, 2, T], BF16, tag="xT", name="xTi")
        ucA = stat.tile([P, 8], F32, tag="uc", name="uc")
        nc.scalar.dma_start(
            out=ucA, in_=dram_view(ap["u"][0, 0], [[1, P], [P, 8]])
        )
        for p in range(8):
            t0 = work.tile([P, D], F32, tag="t0", name="t0", bufs=3)
            nc.vector.scalar_tensor_tensor(
                out=t0, in0=winb, scalar=ucA[:, p : p + 1], in1=binb,
                op0=Alu.mult, op1=Alu.add,
            )
            den, negmd = ln_stats(t0)
            xf[p], xb[p] = finish_x(p, t0, den, negmd, xT, True, fast=(p < 3))
            # interleave the first layer's head-0 encoder with the projection
            if p == 3:
                nxt_xsT = new_xsT(0, 0)
                nxt_qrt = new_qrt(0, 0)
                emit_A(0, 0, nxt_xsT, tchs=(0,), cur_xT=xT)
            elif p == 7:
                emit_A(0, 0, nxt_xsT, tchs=(1,), cur_xT=xT)
                for nt in range(2):
                    emit_R(0, 0, nxt_xsT, nxt_qrt, nt, boundary=True)

        for L in range(NLAYER):
            last = L == NLAYER - 1
            xyT = [
                xypool.tile([P, T], BF16, tag=f"xyT{k}", name=f"xyT{k}_{L}")
                for k in range(8)
            ]
            xsT_h, qrt_h = nxt_xsT, nxt_qrt
            for h in range(NH):
                E = new_E(L, h)
                emit_S(L, h, qrt_h, E)
                xsT_cur = xsT_h
                pipe = h + 1 < NH and VAR != "v0"
                if pipe:
                    xsT_h = new_xsT(L, h + 1)
                    qrt_h = new_qrt(L, h + 1)
                    emit_A(L, h + 1, xsT_h)
                    if VAR == "v1":
                        for nt in range(2):
                            emit_R(L, h + 1, xsT_h, qrt_h, nt)
                    elif VAR == "v2":
                        emit_R(L, h + 1, xsT_h, qrt_h, 0)
                ylnT = work.tile([P, 2, T], BF16, tag="ylnT", name=f"ylnT_{L}{h}")
                for p in range(8):
                    emit_Yp(L, h, E, ylnT, p)
                    if p == 3:
                        emit_V(L, h, ylnT, xsT_cur, xyT, 0)
                        if pipe and VAR == "v2":
                            emit_R(L, h + 1, xsT_h, qrt_h, 1)
                    elif p == 7:
                        emit_V(L, h, ylnT, xsT_cur, xyT, 1)
                        if pipe and VAR == "v3":
                            for nt in range(2):
                                emit_R(L, h + 1, xsT_h, qrt_h, nt)
                if not pipe and h + 1 < NH:
                    xsT_h = new_xsT(L, h + 1)
                    qrt_h = new_qrt(L, h + 1)
                    emit_A(L, h + 1, xsT_h)
                    for nt in range(2):
                        emit_R(L, h + 1, xsT_h, qrt_h, nt)

            # J: decoder matmul + LN(x + LN(yMLP)); the next layer's head-0
            # encoder matmuls are emitted as soon as their xT chunks land.
            new_xT = (
                None
                if last
                else state.tile([P, 2, T], BF16, tag="xT", name=f"xT_{L}")
            )
            new_xf = [None] * 8
            new_xb = [None] * 8
            if not last:
                nxt_xsT = new_xsT(L + 1, 0)
                nxt_qrt = new_qrt(L + 1, 0)
            JVAR = _os.environ.get("BDH_J", "b")
            for p in range(8):
                pm = psY.tile([P, D], F32, tag="y", name=f"pm{L}{p}")
                for k in range(8):
                    nc.tensor.matmul(
                        pm,
                        xyT[k][:, p * P : (p + 1) * P],
                        decS[k],
                        start=(k == 0),
                        stop=(k == 7),
                    )
                fast = p >= 6  # last chunks gate the next layer: low-latency path
                z = work.tile([P, D], F32, tag="z", name=f"z{L}{p}", bufs=3)
                if JVAR == "b":
                    # free the PSUM tile early: zraw = pm - mean right after
                    # stats, then fold den into the residual add (one STT).
                    st = stat.tile([P, 6], F32, tag="st", name="st")
                    nc.vector.bn_stats(out=st, in_=pm)
                    mv = stat.tile([P, 2], F32, tag="mv", name="mv")
                    nc.vector.bn_aggr(out=mv, in_=st)
                    negm = stat.tile([P, 1], F32, tag="negm", name=f"negm{L}{p}")
                    nc.vector.tensor_scalar(
                        out=negm, in0=mv[:, 0:1], scalar1=-1.0, scalar2=None,
                        op0=Alu.mult,
                    )
                    zraw = work.tile([P, D], F32, tag="ln1", name=f"zr_{L}{p}", bufs=3)
                    if p % 2 == 0 and not fast:
                        nc.scalar.activation(
                            out=zraw, in_=pm, func=ACTF.Identity, scale=1.0,
                            bias=negm,
                        )
                    else:
                        nc.vector.tensor_scalar(
                            out=zraw, in0=pm, scalar1=negm, scalar2=None, op0=Alu.add
                        )
                    q = stat.tile([P, 1], F32, tag="q", name="q")
                    nc.scalar.activation(
                        out=q, in_=mv[:, 1:2], func=ACTF.Ln, bias=epsc, scale=1.0
                    )
                    den1 = stat.tile([P, 1], F32, tag="den", name="den")
                    nc.scalar.activation(out=den1, in_=q, func=ACTF.Exp, scale=-0.5)
                    nc.vector.scalar_tensor_tensor(
                        out=z, in0=zraw, scalar=den1, in1=xf[p],
                        op0=Alu.mult, op1=Alu.add,
                    )
                else:
                    den1, negmd1 = ln_stats(pm)
                    ln1 = work.tile([P, D], F32, tag="ln1", name=f"ln1_{L}{p}", bufs=3)
                    if p % 2 == 0 and not fast:
                        nc.scalar.activation(
                            out=ln1, in_=pm, func=ACTF.Identity, scale=den1,
                            bias=negmd1,
                        )
                    else:
                        nc.vector.tensor_scalar(
                            out=ln1, in0=pm, scalar1=den1, scalar2=negmd1,
                            op0=Alu.mult, op1=Alu.add,
                        )
                    zeng = nc.vector if fast else nc.gpsimd
                    zeng.tensor_tensor(out=z, in0=xf[p], in1=ln1, op=Alu.add)
                den2, negmd2 = ln_stats(z)
                new_xf[p], new_xb[p] = finish_x(
                    p, z, den2, negmd2, new_xT, not last, fast=fast
                )
                if not last and VAR != "v0":
                    if p == 3:
                        emit_A(L + 1, 0, nxt_xsT, tchs=(0,), cur_xT=new_xT)
                    elif p == 7:
                        emit_A(L + 1, 0, nxt_xsT, tchs=(1,), cur_xT=new_xT)
                        for nt in range(2):
                            emit_R(L + 1, 0, nxt_xsT, nxt_qrt, nt, boundary=True)
            if not last and VAR == "v0":
                nxt_xsT = new_xsT(L + 1, 0)
                nxt_qrt = new_qrt(L + 1, 0)
                emit_A(L + 1, 0, nxt_xsT, cur_xT=new_xT)
                for nt in range(2):
                    emit_R(L + 1, 0, nxt_xsT, nxt_qrt, nt, boundary=True)
            xf, xb, xT = new_xf, new_xb, new_xT

        # ---- logits (split across DVE/Pool, single output DMA) ----
        lgA = stat.tile([P, 8], F32, tag="lg", name="lg")
        for p in range(8):
            tmp = work.tile([P, D], F32, tag="lgt", name=f"lgt{p}", bufs=4)
            nc.vector.scalar_tensor_tensor(
                out=tmp, in0=xf[p], scalar=1.0, in1=woutb,
                op0=Alu.mult, op1=Alu.mult,
                accum_out=lgA[:, p : p + 1],
            )
        nc.vector.tensor_scalar(
            out=lgA, in0=lgA, scalar1=outbb, scalar2=None, op0=Alu.add
        )
        nc.sync.dma_start(
            out=dram_view(ap["y"][0, 0], [[1, P], [P, 8]]), in_=lgA
        )

    ctx.close()


def _patch_act_tables():
    """All ACT funcs used here (Exp, Ln, Relu, Copy, Identity) live in the
    natural_log_exp_and_others set; empty the others so the table-load pass
    settles on one set and elides every reload (keeps act_func_set ids)."""
    if _CACHE.get("act_patched"):
        return
    import concourse.bacc as bacc
    import concourse.bass_interp as bass_interp

    KEEP = "natural_log_exp_and_others"

    def filtered(orig):
        def f(arch):
            t = orig(arch)
            return {k: (v if k == KEEP else set()) for k, v in t.items()}

        return f

    bacc.get_activation_tables = filtered(bacc.get_activation_tables)
    bass_interp.get_activation_tables = filtered(bass_interp.get_activation_tables)
    _CACHE["act_patched"] = True


def _build(reps=1):
    import concourse.bacc as bacc
    import concourse.tile as tile
    from concourse import mybir

    _patch_act_tables()

    F32 = mybir.dt.float32
    BF16 = mybir.dt.float16

    nc = bacc.Bacc(
        "TRN2",
        target_bir_lowering=False,
        debug=False,
        enable_asserts=True,
        num_devices=8,
    )
    ap = {}
    specs = [
        ("u", [T, 1], F32),
        ("consts", [1, 3 * D + 1], F32),
        ("enc", [NH, D, N], BF16),
        ("encv", [NH, D, N], BF16),
        ("dec", [NH * N, D], BF16),
        ("cost", [N, T], BF16),
        ("sint", [N, T], BF16),
        ("psw", [P, P], BF16),
    ]
    for name, shape, dt in specs:
        ap[name] = nc.dram_tensor(name, shape, dt, kind="ExternalInput").ap()
    ap["y"] = nc.dram_tensor("y", [T, 1], F32, kind="ExternalOutput").ap()

    with tile.TileContext(nc) as tc:
        tc._bdh_reps = reps
        _emit(nc, tc, ap)
    nc.compile()
    return nc


def get_nc(reps=1):
    key = f"nc{reps}"
    if key not in _CACHE:
        _CACHE[key] = _build(reps)
    return _CACHE[key]


def make_in_maps(inputs, in_w, in_b, encoder, encoder_v, decoder, out_w, out_b):
    bf = np.float16
    cosT, sinT = _rope_tables()
    sinT = sinT[np.arange(N) ^ 1]  # pair-swapped: swap folded into the table
    psw = np.zeros((P, P), np.float32)
    psw[np.arange(P), np.arange(P) ^ 1] = 1.0
    consts = np.concatenate(
        [
            in_w.reshape(-1),
            in_b.reshape(-1),
            out_w.reshape(-1),
            out_b.reshape(-1),
        ]
    ).astype(np.float32).reshape(1, 3 * D + 1)
    common = {
        "consts": np.ascontiguousarray(consts),
        "enc": np.ascontiguousarray(encoder).astype(bf),
        "encv": np.ascontiguousarray(encoder_v).astype(bf),
        "dec": np.ascontiguousarray(decoder).astype(bf),
        "cost": cosT.astype(bf),
        "sint": sinT.astype(bf),
        "psw": psw.astype(bf),
    }
    return [
        {"u": np.ascontiguousarray(inputs[b].reshape(T, 1)).astype(np.float32), **common}
        for b in range(B)
    ]


def get_runner(reps=1):
    """Cached jitted shard_map runner over 8 cores (mirrors
    bass2jax.run_bass_via_pjrt's multi-core path, but reusable across calls)."""
    key = f"runner{reps}"
    if key in _CACHE:
        return _CACHE[key]
    import jax
    from jax.experimental.shard_map import shard_map
    from jax.sharding import Mesh, PartitionSpec

    from concourse import mybir
    from concourse.bass2jax import (
        _bass_exec_p,
        install_neuronx_cc_hook,
        partition_id_tensor,
    )

    nc = get_nc(reps)
    install_neuronx_cc_hook()

    partition_name = nc.partition_id_tensor.name if nc.partition_id_tensor else None
    in_names, out_names, out_avals, zero_outs = [], [], [], []
    for alloc in nc.m.functions[0].allocations:
        if not isinstance(alloc, mybir.MemoryLocationSet):
            continue
        name = alloc.memorylocations[0].name
        if alloc.kind == "ExternalInput":
            if name != partition_name:
                in_names.append(name)
        elif alloc.kind == "ExternalOutput":
            shape = tuple(alloc.tensor_shape)
            dtype = mybir.dt.np(alloc.dtype)
            out_names.append(name)
            out_avals.append(jax.core.ShapedArray(shape, dtype))
            zero_outs.append(np.zeros(shape, dtype))
    n_params = len(in_names)
    all_in_names = in_names + out_names
    if partition_name is not None:
        all_in_names = all_in_names + [partition_name]
    donate = tuple(range(n_params, n_params + len(out_names)))

    def _body(*args):
        operands = list(args)
        if partition_name is not None:
            operands.append(partition_id_tensor())
        outs = _bass_exec_p.bind(
            *operands,
            out_avals=tuple(out_avals),
            in_names=tuple(all_in_names),
            out_names=tuple(out_names),
            lowering_input_output_aliases=(),
            sim_require_finite=True,
            sim_require_nnan=True,
            nc=nc,
        )
        return tuple(outs)

    devices = jax.devices()[:B]
    mesh = Mesh(np.asarray(devices), ("core",))
    in_specs = (PartitionSpec("core"),) * (n_params + len(out_names))
    out_specs = (PartitionSpec("core"),) * len(out_names)
    sharded = jax.jit(
        shard_map(
            _body, mesh=mesh, in_specs=in_specs, out_specs=out_specs, check_rep=False
        ),
        donate_argnums=donate,
        keep_unused=True,
    )

    runner = {
        "sharded": sharded,
        "in_names": in_names,
        "out_names": out_names,
        "zero_outs": zero_outs,
        "n_params": n_params,
        "mesh": mesh,
    }
    _CACHE[key] = runner
    return runner


def run_on_device(in_maps, iters=1):
    """Run the kernel `iters` times; returns (list of per-core out dicts,
    per-iteration wall seconds over the last iters-1 runs or the single run)."""
    import time

    import jax

    r = get_runner()
    concat_in = [
        np.concatenate([np.asarray(m[name]) for m in in_maps], axis=0)
        for name in r["in_names"]
    ]
    concat_in = [jax.device_put(a) for a in concat_in]
    for a in concat_in:
        a.block_until_ready()

    def one_call():
        zeros = [
            np.zeros((B * z.shape[0], *z.shape[1:]), z.dtype) for z in r["zero_outs"]
        ]
        return r["sharded"](*concat_in, *zeros)

    outs = one_call()  # compile + first run
    for o in outs:
        o.block_until_ready()
    per_iter = None
    if iters > 1:
        t0 = time.perf_counter()
        for _ in range(iters - 1):
            outs = one_call()
        for o in outs:
            o.block_until_ready()
        per_iter = (time.perf_counter() - t0) / (iters - 1)
    results = []
    for c in range(B):
        d = {}
        for i, name in enumerate(r["out_names"]):
            full = np.asarray(outs[i])
            pershape = r["zero_outs"][i].shape
            d[name] = full.reshape(B, *pershape)[c]
        results.append(d)
    return results, per_iter


def kernel(inputs, in_w, in_b, encoder, encoder_v, decoder, out_w, out_b):
    inputs = np.asarray(inputs)
    in_maps = make_in_maps(
        np.asarray(inputs, np.float32),
        np.asarray(in_w, np.float32),
        np.asarray(in_b, np.float32),
        np.asarray(encoder, np.float32),
        np.asarray(encoder_v, np.float32),
        np.asarray(decoder, np.float32),
        np.asarray(out_w, np.float32),
        np.asarray(out_b, np.float32),
    )
    results, _ = run_on_device(in_maps, iters=1)
    out = np.stack([results[b]["y"] for b in range(B)], axis=0)  # (8, 1024, 1)
    return out.astype(np.float32)


if __name__ == "__main__":
    rng = np.random.default_rng(0)
    out = kernel(
        inputs=rng.standard_normal((B, T), dtype=np.float32),
        in_w=rng.standard_normal((D, 1), dtype=np.float32) * 0.02,
        in_b=np.zeros((D,), np.float32),
        encoder=rng.standard_normal((NH, D, N), dtype=np.float32) * 0.02,
        encoder_v=rng.standard_normal((NH, D, N), dtype=np.float32) * 0.02,
        decoder=rng.standard_normal((NH * N, D), dtype=np.float32) * 0.02,
        out_w=rng.standard_normal((1, D), dtype=np.float32) * 0.02,
        out_b=np.zeros((1,), np.float32),
    )
    print("out", out.shape, out.dtype, np.abs(out).max())
